# revision 22
# baseline (speedup 1.0000x reference)
"""Trainium2 Bass kernel for causal multi-head attention (prefill).

Problem: x[2,2048,768], 12 heads x 64 dim, causal softmax(QK^T/8)V + out-proj.

Sharding (8 cores, no collectives): core c handles batch c//4 and head group
c%4 (3 heads).  Each core computes, for its batch b and heads hs:
    qT,kT = (Wq_hs @ x_b^T), (Wk_hs @ x_b^T)        [192, 2048] (transposed)
    v     = x_b @ Wv_hs^T                            [2048, 192]
    expT  = exp(scoresT/8) masked causally           [kv, sq] per head
    ctxT_h = v_aug^T @ expT  (extra row = softmax denom via ones column)
    outT_partial = Wo[:,cols_hs] @ (ctxT/den)        [768, 2048]
Host sums the 4 partial outputs per batch and transposes back.

v7: the PE p-state halves the clock for ~3us after ANY idle gap, so the
design centers on a gap-free PE stream.  PSUM accumulations are strictly
bank-exclusive (a psum "zero region" = one 2KB bank; start=True lazily
zeroes the whole bank, so co-tenant accumulations corrupt each other - a
same-bank concurrent matmul pair even faults the device).  Budget: 8 banks.
  - each 512-wide q window runs in TWO passes: pass1 = heads 0+1 with a
    DOUBLE-buffered [128,2,512] scores tile (2 banks x 2 bufs), pass2 =
    head 2 with [128,512] x 2 bufs from the same pool.  Double buffering
    means scores(i+1) overlaps exp(i): the PE never waits on ACT.
  - ctx psums: pass1 [128,2,512] (2 banks), pass2 [128,512] reusing the
    same 2-bank pool buffer after the pass1 norms read it out.
  - one merged ACT exp instruction per (pass, kv tile): [128,2,512-c0] /
    [128,512-c0] (multi-bank ACT reads HW-validated).
  - causal mask via a PE bias matmul inside ONE accumulation group:
    bias writes -30000*tri into [c0:c0+128] (start=True pending-zeroes
    the bank), score r0 accumulates there, score r1 [c0+128:512] lands on
    pending-zero bytes (= fresh write) and closes the group.  HW exp
    underflows exp(0.125*(s-30000)) to exact 0 (microbenched).
  - score matmuls are K=64: pass1 pairs h0(g0)||h1(g64); pass2 pipelines
    TWO kv tiles (pch bufs) and pairs them g0||g64 (h2's q/k dual-copied).
  - h1's v_aug weight is reversed ([ones@0|zeros|v]) so its ctx lands in
    psum rows 64:128 (den at row 0): normalize writes ctxT01[64:128]
    directly.
  - out-proj per window in mt-pairs: two K=128 wo01 matmuls then the two
    K=64 wo2 matmuls as a concurrent g0/g64 pair (different banks).
  - proj/outproj work is emitted as fine-grained filler units inside the
    chain loops so the dynamic Tile scheduler always has ready PE work.
  - bf16 datapath; PSUM + softmax normalization fp32.
"""

import numpy as np

import concourse.bass as bass
import concourse.tile as tile
from concourse import bacc, mybir
from concourse.bass_utils import run_bass_kernel_spmd

F32 = mybir.dt.float32
BF16 = mybir.dt.bfloat16

B, S, D = 2, 2048, 768
H, DH = 12, 64
HPC = 3                 # heads per core
GH = HPC * DH           # 192 head dims per core
NCORES = 8
KT = D // 128           # 6 contraction tiles for projections
WJ = 512                # q window width
NJ = S // WJ            # 4 windows
NKV = S // 128          # 16 kv tiles of 128
NEG = -30000.0          # causal bias: exp(0.125*(s+NEG)) == 0 (HW-checked)


def build():
    nc = bacc.Bacc("TRN2", target_bir_lowering=False, debug=False)

    # host pre-packed input: per partition line p:
    #   [ x (4 nt-blocks x 6 k x 512) | wq 6x128 | wk 6x128 | wqk2 6x128
    #     | wv 6x192 ]  (wqk2 cols 0:64 = Wk2^T, 64:128 = Wq2^T)
    XW = 4 * KT * 512 + 3 * KT * 128 + KT * 192
    xw = nc.dram_tensor("xw", [128, XW], BF16, kind="ExternalInput")
    # wo[:, 0, :] = Wo cols (head rows) 0:128; wo[:, 1, :] = rows 128:192
    # duplicated twice (p and p-64 hold the same row) for g0/g64 pairing.
    wo = nc.dram_tensor("wo", [128, 2, D], BF16, kind="ExternalInput")
    # misc[:, 0:128] = trib (NEG above diagonal, [sq, kv] layout);
    # misc[:, 128:256] = identity.
    misc = nc.dram_tensor("misc", [128, 256], BF16, kind="ExternalInput")
    outT = nc.dram_tensor("outT", [D, S], F32, kind="ExternalOutput")

    with tile.TileContext(nc) as tc, \
         nc.allow_low_precision(reason="bf16 datapath, fp32 psum/normalize"):
        with tc.tile_pool(name="sb", bufs=1) as sb, \
             tc.tile_pool(name="sbe", bufs=3) as sbe, \
             tc.tile_pool(name="sbf", bufs=4) as sbf, \
             tc.tile_pool(name="sbo", bufs=3) as sbo, \
             tc.tile_pool(name="sbn", bufs=3) as sbn, \
             tc.tile_pool(name="ps", bufs=2, space="PSUM") as ps, \
             tc.tile_pool(name="pch", bufs=2, space="PSUM") as pch, \
             tc.tile_pool(name="psc", bufs=1, space="PSUM") as psc:

            # ---- phase 0: load weights + x ----
            xsb = sb.tile([128, 4, KT, 512], BF16, tag="xsb")
            wqkx_sb = sb.tile([128, 3, KT, 128], BF16, tag="wqkx")
            wv_sb = sb.tile([128, KT, 192], BF16, tag="wv")
            misc_sb = sb.tile([128, 256], BF16, tag="misc")
            XOFF = 4 * KT * 512
            nc.sync.dma_start(misc_sb, misc[:, :])
            nc.sync.dma_start(
                wqkx_sb,
                xw[:, XOFF:XOFF + 3 * KT * 128].rearrange(
                    "p (w k m) -> p w k m", w=3, k=KT))
            xw_x = xw[:, 0:XOFF].rearrange("p (t k n) -> p t k n", t=4, k=KT)
            # nt0 split per-k so the first chain starts as k0 lands
            for k in range(KT):
                nc.sync.dma_start(xsb[:, 0, k], xw_x[:, 0, k])
            nc.sync.dma_start(
                wv_sb,
                xw[:, XOFF + 3 * KT * 128:XW].rearrange(
                    "p (k m) -> p k m", k=KT))
            for nt in range(1, 4):
                nc.sync.dma_start(xsb[:, nt], xw_x[:, nt])
            wo_sb = sb.tile([128, 2, D], BF16, tag="wo")
            nc.sync.dma_start(wo_sb, wo[:, :, :])
            trib_sb = misc_sb[:, 0:128]
            i128_sb = misc_sb[:, 128:256]
            wq_sb, wk_sb, wqk2_sb = (wqkx_sb[:, 0], wqkx_sb[:, 1],
                                     wqkx_sb[:, 2])
            wo01_sb = wo_sb[:, 0, :]
            wo2d_sb = wo_sb[:, 1, :]
            # preload the exp ACT table set (~1.3us) while DMA streams
            warm = sbn.tile([128, 64], F32, tag="warm", name="warm")
            nc.scalar.activation(warm[0:1, 0:2], misc_sb[0:1, 0:2],
                                 mybir.ActivationFunctionType.Exp, scale=1.0)

            # ---- phase 1: projections ----
            qt_sb = sb.tile([128, S], BF16, tag="qt")
            kt_sb = sb.tile([128, S], BF16, tag="kt")
            q2d = sb.tile([128, S], BF16, tag="q2d")
            k2d = sb.tile([128, S], BF16, tag="k2d")

            def _qk_casts(w, ntw, pp):
                if w == 0:
                    nc.vector.tensor_copy(qt_sb[:, ntw], pp)
                elif w == 1:
                    nc.vector.tensor_copy(kt_sb[:, ntw], pp)
                else:
                    nc.vector.tensor_copy(k2d[0:64, ntw], pp[0:64, :])
                    nc.vector.tensor_copy(q2d[64:128, ntw], pp[64:128, :])
                    nc.sync.dma_start(k2d[64:128, ntw], k2d[0:64, ntw])
                    nc.sync.dma_start(q2d[0:64, ntw], q2d[64:128, ntw])

            def proj_qk0():
                # 3 psum chains (3 bank-exclusive slots) accumulate per
                # arriving x k-tile so the PE starts ~2us into the DMA
                pa = pch.tile([128, 2, 512], F32, tag="sp", name="p0a")
                pb = pch.tile([128, 512], F32, tag="sp", name="p0b")
                slots = (pa[:, 0, :], pa[:, 1, :], pb)
                trips = (wq_sb, wk_sb, wqk2_sb)
                for k in range(KT):
                    for w, wsb in enumerate(trips):
                        nc.tensor.matmul(slots[w], wsb[:, k, :],
                                         xsb[:, 0, k, :],
                                         start=(k == 0), stop=(k == KT - 1))
                for w in range(3):
                    _qk_casts(w, slice(0, 512), slots[w])

            def filler_qk_units(nt):
                """proj_qk(nt) as 3 filler units (one 6-matmul chain each)."""
                ntw = slice(nt * 512, (nt + 1) * 512)

                def chain(w):
                    def f():
                        pp = ps.tile([128, 512], F32, tag="sc",
                                     name=f"pqk{nt}_{w}")
                        wsb = (wq_sb, wk_sb, wqk2_sb)[w]
                        for k in range(KT):
                            nc.tensor.matmul(pp, wsb[:, k, :],
                                             xsb[:, nt, k, :],
                                             start=(k == 0), stop=(k == KT - 1))
                        _qk_casts(w, ntw, pp)
                    return f
                return [(1.3, chain(w)) for w in range(3)]

            # v_aug: [128, NKV, 384]; head 0/2 at 128h: [v(64)|ones|zeros63];
            # head 1 reversed: [ones@0|zeros(63)|v(64)] so ctx1 lands in
            # psum rows 64:128 and den1 at row 0.
            vaug = sb.tile([128, NKV, 384], BF16, tag="vaug")
            for h in (0, 2):
                nc.vector.memset(vaug[:, :, 128 * h + 65:128 * (h + 1)], 0.0)
                nc.scalar.activation(
                    vaug[:, :, 128 * h + 64:128 * h + 65],
                    trib_sb[:, h * NKV:(h + 1) * NKV].rearrange(
                        "p (t c) -> p t c", c=1),
                    mybir.ActivationFunctionType.Copy, bias=1.0, scale=0.0)
            nc.vector.memset(vaug[:, :, 128 + 1:128 + 64], 0.0)
            nc.scalar.activation(
                vaug[:, :, 128 + 0:128 + 1],
                trib_sb[:, NKV:2 * NKV].rearrange("p (t c) -> p t c", c=1),
                mybir.ActivationFunctionType.Copy, bias=1.0, scale=0.0)

            def proj_v(i):
                """one filler unit: 6 matmuls of 192 cols + 2 casts
                (pp cols 0:64 -> v0; pp cols 64:192 -> v1|v2 contiguous)."""
                pp = ps.tile([128, 192], F32, tag="sc", name=f"pv{i}")
                c0 = (i % 4) * 128
                for k in range(KT):
                    nc.tensor.matmul(
                        pp,
                        xsb[:, i // 4, k, c0:c0 + 128],
                        wv_sb[:, k, :],
                        start=(k == 0), stop=(k == KT - 1))
                nc.vector.tensor_copy(vaug[:, i, 0:64], pp[:, 0:64])
                nc.vector.tensor_copy(vaug[:, i, 192:320], pp[:, 64:192])

            # ---- phase 2/3 helpers ----
            def kslc(h, i, g=None):
                if h == 0:
                    return kt_sb[0:64, i * 128:(i + 1) * 128]
                if h == 1:
                    return kt_sb[64:128, i * 128:(i + 1) * 128]
                return k2d[g:g + 64, i * 128:(i + 1) * 128]

            def qslc(h, c0, c1, g=None):
                if h == 0:
                    return qt_sb[0:64, c0:c1]
                if h == 1:
                    return qt_sb[64:128, c0:c1]
                return q2d[g:g + 64, c0:c1]

            ctxT01 = sb.tile([128, S], BF16, tag="ctxT01")
            ctxT2 = sb.tile([128, S], BF16, tag="ctxT2")

            def scores_pair(slots, J):
                """Paired bias+score emission for a list of (spt_h, h, i, g)
                covering DISTINCT psum banks.  One accumulation group per
                bank: bias starts it (pending-zeroing the bank), r0
                accumulates onto the bias, r1 lands on pending-zero bytes
                (= fresh write) and closes the group.  Emission interleaves
                the banks so the K=64 score mms pair on opposite row
                groups."""
                base = WJ * J
                diag = [s for s in slots if 128 * s[2] - base >= 0]
                full = [s for s in slots if 128 * s[2] - base < 0]
                for spt_h, h, i, g in full:
                    nc.tensor.matmul(spt_h, kslc(h, i, g=g),
                                     qslc(h, base, base + WJ, g=g),
                                     start=True, stop=True)
                for spt_h, h, i, g in diag:
                    c0 = 128 * i - base
                    nc.tensor.matmul(spt_h[:, c0:c0 + 128], trib_sb,
                                     i128_sb, start=True, stop=False)
                for spt_h, h, i, g in diag:
                    c0 = 128 * i - base
                    r1 = c0 + 128
                    nc.tensor.matmul(spt_h[:, c0:r1], kslc(h, i, g=g),
                                     qslc(h, base + c0, base + r1, g=g),
                                     start=False, stop=(r1 >= WJ))
                for spt_h, h, i, g in diag:
                    r1 = 128 * i - base + 128
                    if r1 < WJ:
                        nc.tensor.matmul(spt_h[:, r1:WJ], kslc(h, i, g=g),
                                         qslc(h, base + r1, base + WJ, g=g),
                                         start=False, stop=True)

            def chains01(J, fillers):
                """pass 1: heads 0+1, double-buffered scores."""
                ctx01 = psc.tile([128, 2, WJ], F32, tag="ctx",
                                 name=f"ctx01_{J}")
                imax = 4 * J + 3
                base = WJ * J

                def emit_ctx(i, esb2):
                    c0 = max(0, 128 * i - base)
                    for h in range(2):
                        nc.tensor.matmul(
                            ctx01[:, h, c0:WJ],
                            vaug[:, i, 128 * h:128 * h + 128],
                            esb2[:, h, c0:WJ],
                            start=(i == 0), stop=(i == imax))

                prev = None
                budget = 0.0
                for i in range(imax + 1):
                    c0 = max(0, 128 * i - base)
                    spt = pch.tile([128, 2, WJ], F32, tag="sp", name="spt2")
                    # h0 in bank0 (g0) || h1 in bank1 (g64)
                    scores_pair([(spt[:, 0, :], 0, i, None),
                                 (spt[:, 1, :], 1, i, None)], J)
                    esb2 = sbe.tile([128, 2, WJ], BF16, tag="exp2",
                                    name="esb2")
                    nc.scalar.activation(
                        esb2[:, :, c0:WJ], spt[:, :, c0:WJ],
                        mybir.ActivationFunctionType.Exp, scale=0.125)
                    if prev is not None:
                        emit_ctx(i - 1, prev)
                    prev = esb2
                    budget += 0.35
                    while fillers and budget >= fillers[0][0]:
                        cost, fn = fillers.pop(0)
                        budget -= cost
                        fn()
                emit_ctx(imax, prev)
                return ctx01

            def chains2(J, fillers):
                """pass 2: head 2, TWO kv tiles pipelined per step (the two
                pch bufs), paired g0||g64 via the dual q2d/k2d copies."""
                ctx2 = psc.tile([128, WJ], F32, tag="ctx", name=f"ctx2_{J}")
                imax = 4 * J + 3
                base = WJ * J

                def emit_ctx(i, esb1):
                    c0 = max(0, 128 * i - base)
                    nc.tensor.matmul(
                        ctx2[:, c0:WJ],
                        vaug[:, i, 256:384],
                        esb1[:, c0:WJ],
                        start=(i == 0), stop=(i == imax))

                prevs = []
                budget = 0.0
                for t in range((imax + 1) // 2):
                    i0, i1 = 2 * t, 2 * t + 1
                    sp0 = pch.tile([128, WJ], F32, tag="sp", name="spt1a")
                    sp1 = pch.tile([128, WJ], F32, tag="sp", name="spt1b")
                    scores_pair([(sp0, 2, i0, 0), (sp1, 2, i1, 64)], J)
                    for i, sp in ((i0, sp0), (i1, sp1)):
                        c0 = max(0, 128 * i - base)
                        esb1 = sbf.tile([128, WJ], BF16, tag="exp1",
                                        name="esb1")
                        nc.scalar.activation(
                            esb1[:, c0:WJ], sp[:, c0:WJ],
                            mybir.ActivationFunctionType.Exp, scale=0.125)
                        prevs.append((i, esb1))
                    while len(prevs) > 2:
                        emit_ctx(*prevs.pop(0))
                    budget += 0.45
                    while fillers and budget >= fillers[0][0]:
                        cost, fn = fillers.pop(0)
                        budget -= cost
                        fn()
                for pe in prevs:
                    emit_ctx(*pe)
                return ctx2

            def norms01(J, ctx01):
                w = slice(WJ * J, WJ * (J + 1))
                denT = sbn.tile([128, 2, WJ], BF16, tag="den", name="denT")
                nc.vector.tensor_copy(denT[64:65, 0, :], ctx01[64:65, 0, :])
                nc.vector.tensor_copy(denT[0:1, 0, :], ctx01[0:1, 1, :])
                bps0 = ps.tile([64, WJ], F32, tag="sc", name="bps0")
                nc.tensor.matmul(bps0, onesb[64:65, 0:64],
                                 denT[64:65, 0, :], start=True, stop=True)
                bps1 = ps.tile([64, WJ], F32, tag="sc", name="bps1")
                nc.tensor.matmul(bps1, onesb[0:1, 0:64],
                                 denT[0:1, 0, :], start=True, stop=True)
                inv0 = sbn.tile([64, WJ], F32, tag="inv", name="inv0")
                nc.vector.reciprocal_approx_fast(inv0, bps0)
                inv1 = sbn.tile([128, WJ], F32, tag="inv1", name="inv1")
                nc.vector.reciprocal_approx_fast(inv1[0:64, :], bps1)
                nc.sync.dma_start(inv1[64:128, :], inv1[0:64, :])
                nc.vector.tensor_mul(ctxT01[0:64, w], ctx01[0:64, 0, :],
                                     inv0)
                nc.vector.tensor_mul(ctxT01[64:128, w], ctx01[64:128, 1, :],
                                     inv1[64:128, :])

            def norms2(J, ctx2):
                w = slice(WJ * J, WJ * (J + 1))
                denT = sbn.tile([128, WJ], BF16, tag="den2", name="denT2")
                nc.vector.tensor_copy(denT[64:65, :], ctx2[64:65, :])
                bps2 = ps.tile([64, WJ], F32, tag="sc", name="bps2")
                nc.tensor.matmul(bps2, onesb[64:65, 0:64],
                                 denT[64:65, :], start=True, stop=True)
                inv2 = sbn.tile([64, WJ], F32, tag="inv2", name="inv2")
                nc.vector.reciprocal_approx_fast(inv2, bps2)
                nc.vector.tensor_mul(ctxT2[0:64, w], ctx2[0:64, :], inv2)
                # replicate ctxT2 into rows 64:128 for the g64 outproj pair
                nc.sync.dma_start(ctxT2[64:128, w], ctxT2[0:64, w])

            def outproj_units(J):
                """3 filler units; unit p = mt pair (2p, 2p+1): two K=128
                wo01 matmuls then the K=64 wo2 matmuls as a g0||g64 pair."""
                w = slice(WJ * J, WJ * (J + 1))

                def unit(p):
                    def f():
                        mta, mtb = 2 * p, 2 * p + 1
                        pa = ps.tile([128, WJ], F32, tag="sc", name="opA")
                        pb = ps.tile([128, WJ], F32, tag="sc", name="opB")
                        nc.tensor.matmul(
                            pa, wo01_sb[:, mta * 128:(mta + 1) * 128],
                            ctxT01[:, w], start=True, stop=False)
                        nc.tensor.matmul(
                            pb, wo01_sb[:, mtb * 128:(mtb + 1) * 128],
                            ctxT01[:, w], start=True, stop=False)
                        nc.tensor.matmul(
                            pa, wo2d_sb[0:64, mta * 128:(mta + 1) * 128],
                            ctxT2[0:64, w], start=False, stop=True)
                        nc.tensor.matmul(
                            pb, wo2d_sb[64:128, mtb * 128:(mtb + 1) * 128],
                            ctxT2[64:128, w], start=False, stop=True)
                        for mt, pp in ((mta, pa), (mtb, pb)):
                            osb = sbo.tile([128, WJ], F32, tag="osb",
                                           name="osb")
                            nc.vector.tensor_copy(osb, pp)
                            nc.sync.dma_start(
                                outT[mt * 128:(mt + 1) * 128, w], osb)
                    return f
                return [(1.1, unit(p)) for p in range(3)]

            # ones rows for the denominator broadcast (rows 0/64)
            onesb = sbn.tile([128, 64], BF16, tag="onesb", name="onesb")
            for r in (0, 64):
                nc.scalar.activation(
                    onesb[r:r + 1, :],
                    trib_sb[r:r + 1, 0:64],
                    mybir.ActivationFunctionType.Copy, bias=1.0, scale=0.0)

            # ---- interleaved schedule ----
            # qk(nt) is needed by window nt's pass1 - its filler units run
            # in window nt-1 and are flushed at the boundary.  proj_v(i)
            # must be emitted before the ctx matmuls that read vaug[i].
            proj_qk0()
            for i in range(4):
                proj_v(i)
            fq1 = filler_qk_units(1)
            c01 = chains01(0, fq1)
            norms01(0, c01)
            c2 = chains2(0, fq1)
            for cost, fn in fq1:
                fn()
            norms2(0, c2)
            for i in range(4, 8):
                proj_v(i)

            fq2 = filler_qk_units(2) + outproj_units(0)
            c01 = chains01(1, fq2)
            norms01(1, c01)
            c2 = chains2(1, fq2)
            for cost, fn in fq2:
                fn()
            norms2(1, c2)
            for i in range(8, 12):
                proj_v(i)

            fq3 = filler_qk_units(3) + outproj_units(1)
            c01 = chains01(2, fq3)
            norms01(2, c01)
            c2 = chains2(2, fq3)
            for cost, fn in fq3:
                fn()
            norms2(2, c2)
            for i in range(12, 16):
                proj_v(i)

            fq4 = outproj_units(2)
            c01 = chains01(3, fq4)
            norms01(3, c01)
            c2 = chains2(3, fq4)
            for cost, fn in fq4:
                fn()
            norms2(3, c2)
            for cost, fn in outproj_units(3):
                fn()

    nc.compile()
    return nc


def shard_inputs(x, Wq, Wk, Wv, Wo):
    import ml_dtypes
    bf16 = ml_dtypes.bfloat16

    def krearrange(wT, cols):
        # [D, cols] -> [128, KT*cols]; line p holds wT[k*128+p, :] for all k
        return np.ascontiguousarray(
            wT.reshape(KT, 128, cols).transpose(1, 0, 2).reshape(128, KT * cols)
        ).astype(bf16)

    x = np.asarray(x, np.float32)
    trib = (np.triu(np.ones((128, 128), np.float32), 1) * NEG).astype(bf16)
    i128 = np.eye(128, dtype=np.float32).astype(bf16)
    misc_host = np.concatenate([trib, i128], axis=1)
    in_maps = []
    for c in range(NCORES):
        b, g = c // 4, c % 4
        rs = slice(GH * g, GH * g + GH)
        wqT = np.ascontiguousarray(np.asarray(Wq, np.float32)[rs].T)  # [D,192]
        wkT = np.ascontiguousarray(np.asarray(Wk, np.float32)[rs].T)
        wqk2 = np.concatenate([wkT[:, 128:192], wqT[:, 128:192]], axis=1)
        wv_t = np.ascontiguousarray(np.asarray(Wv, np.float32)[rs].T)  # [D,192]
        xT = np.ascontiguousarray(x[b].T)                     # [D, S]
        xb = xT.reshape(KT, 128, S)
        xparts = [np.ascontiguousarray(
                      xb[:, :, nt * 512:(nt + 1) * 512]
                  ).transpose(1, 0, 2).reshape(128, KT * 512)
                  for nt in range(4)]
        xw_host = np.concatenate(
            xparts + [krearrange(np.ascontiguousarray(wqT[:, 0:128]), 128),
                      krearrange(np.ascontiguousarray(wkT[:, 0:128]), 128),
                      krearrange(np.ascontiguousarray(wqk2), 128),
                      krearrange(wv_t, 192)], axis=1)
        woT = np.asarray(Wo, np.float32)[:, rs].T             # [192, 768]
        wo01 = woT[0:128]
        wo2d = np.concatenate([woT[128:192], woT[128:192]], axis=0)  # [128,768]
        wo_host = np.stack([wo01, wo2d], axis=1)              # [128, 2, 768]
        in_maps.append({
            "xw": np.ascontiguousarray(xw_host).astype(bf16),
            "wo": np.ascontiguousarray(wo_host).astype(bf16),
            "misc": misc_host,
        })
    return in_maps


def assemble(results, bo):
    out = np.zeros((B, S, D), np.float32)
    for c in range(NCORES):
        out[c // 4] += results[c]["outT"].T
    return out + np.asarray(bo, np.float32)[None, None, :]


_NC = None


def kernel(x, Wq, Wk, Wv, Wo, bo, **run_kwargs):
    global _NC
    if _NC is None:
        _NC = build()
    in_maps = shard_inputs(x, Wq, Wk, Wv, Wo)
    res = run_bass_kernel_spmd(_NC, in_maps, core_ids=list(range(NCORES)),
                               **run_kwargs)
    out = assemble(res.results, bo)
    kernel.last_results = res
    return out


# revision 23
# speedup vs baseline: 1.0047x; 1.0047x over previous
"""Trainium2 Bass kernel for causal multi-head attention (prefill).

Problem: x[2,2048,768], 12 heads x 64 dim, causal softmax(QK^T/8)V + out-proj.

Sharding (8 cores, no collectives): core c handles batch c//4 and head group
c%4 (3 heads).  Each core computes, for its batch b and heads hs:
    qT,kT = (Wq_hs @ x_b^T), (Wk_hs @ x_b^T)        [192, 2048] (transposed)
    v     = x_b @ Wv_hs^T                            [2048, 192]
    expT  = exp(scoresT/8) masked causally           [kv, sq] per head
    ctxT_h = v_aug^T @ expT  (extra row = softmax denom via ones column)
    outT_partial = Wo[:,cols_hs] @ (ctxT/den)        [768, 2048]
Host sums the 4 partial outputs per batch and transposes back.

v7: the PE p-state halves the clock for ~3us after ANY idle gap, so the
design centers on a gap-free PE stream.  PSUM accumulations are strictly
bank-exclusive (a psum "zero region" = one 2KB bank; start=True lazily
zeroes the whole bank, so co-tenant accumulations corrupt each other - a
same-bank concurrent matmul pair even faults the device).  Budget: 8 banks.
  - each 512-wide q window runs in TWO passes: pass1 = heads 0+1 with a
    DOUBLE-buffered [128,2,512] scores tile (2 banks x 2 bufs), pass2 =
    head 2 with [128,512] x 2 bufs from the same pool.  Double buffering
    means scores(i+1) overlaps exp(i): the PE never waits on ACT.
  - ctx psums: pass1 [128,2,512] (2 banks), pass2 [128,512] reusing the
    same 2-bank pool buffer after the pass1 norms read it out.
  - one merged ACT exp instruction per (pass, kv tile): [128,2,512-c0] /
    [128,512-c0] (multi-bank ACT reads HW-validated).
  - causal mask via a PE bias matmul inside ONE accumulation group:
    bias writes -30000*tri into [c0:c0+128] (start=True pending-zeroes
    the bank), score r0 accumulates there, score r1 [c0+128:512] lands on
    pending-zero bytes (= fresh write) and closes the group.  HW exp
    underflows exp(0.125*(s-30000)) to exact 0 (microbenched).
  - score matmuls are K=64: pass1 pairs h0(g0)||h1(g64); pass2 pipelines
    TWO kv tiles (pch bufs) and pairs them g0||g64 (h2's q/k dual-copied).
  - h1's v_aug weight is reversed ([ones@0|zeros|v]) so its ctx lands in
    psum rows 64:128 (den at row 0): normalize writes ctxT01[64:128]
    directly.
  - out-proj per window in mt-pairs: two K=128 wo01 matmuls then the two
    K=64 wo2 matmuls as a concurrent g0/g64 pair (different banks).
  - proj/outproj work is emitted as fine-grained filler units inside the
    chain loops so the dynamic Tile scheduler always has ready PE work.
  - bf16 datapath; PSUM + softmax normalization fp32.
"""

import numpy as np

import concourse.bass as bass
import concourse.tile as tile
from concourse import bacc, mybir
from concourse.bass_utils import run_bass_kernel_spmd

F32 = mybir.dt.float32
BF16 = mybir.dt.bfloat16

B, S, D = 2, 2048, 768
H, DH = 12, 64
HPC = 3                 # heads per core
GH = HPC * DH           # 192 head dims per core
NCORES = 8
KT = D // 128           # 6 contraction tiles for projections
WJ = 512                # q window width
NJ = S // WJ            # 4 windows
NKV = S // 128          # 16 kv tiles of 128
NEG = -30000.0          # causal bias: exp(0.125*(s+NEG)) == 0 (HW-checked)


def build():
    nc = bacc.Bacc("TRN2", target_bir_lowering=False, debug=False)

    # host pre-packed input: per partition line p:
    #   [ x (4 nt-blocks x 6 k x 512) | wq 6x128 | wk 6x128 | wqk2 6x128
    #     | wv 6x192 ]  (wqk2 cols 0:64 = Wk2^T, 64:128 = Wq2^T)
    XW = 4 * KT * 512 + 3 * KT * 128 + KT * 192
    xw = nc.dram_tensor("xw", [128, XW], BF16, kind="ExternalInput")
    # wo[:, 0, :] = Wo cols (head rows) 0:128; wo[:, 1, :] = rows 128:192
    # duplicated twice (p and p-64 hold the same row) for g0/g64 pairing.
    wo = nc.dram_tensor("wo", [128, 2, D], BF16, kind="ExternalInput")
    # misc[:, 0:128] = trib (NEG above diagonal, [sq, kv] layout);
    # misc[:, 128:256] = identity.
    misc = nc.dram_tensor("misc", [128, 256], BF16, kind="ExternalInput")
    outT = nc.dram_tensor("outT", [D, S], F32, kind="ExternalOutput")

    with tile.TileContext(nc) as tc, \
         nc.allow_low_precision(reason="bf16 datapath, fp32 psum/normalize"):
        with tc.tile_pool(name="sb", bufs=1) as sb, \
             tc.tile_pool(name="sbe", bufs=3) as sbe, \
             tc.tile_pool(name="sbf", bufs=4) as sbf, \
             tc.tile_pool(name="sbo", bufs=3) as sbo, \
             tc.tile_pool(name="sbn", bufs=3) as sbn, \
             tc.tile_pool(name="ps", bufs=2, space="PSUM") as ps, \
             tc.tile_pool(name="pch", bufs=2, space="PSUM") as pch, \
             tc.tile_pool(name="psc", bufs=1, space="PSUM") as psc:

            # ---- phase 0: load weights + x ----
            xsb = sb.tile([128, 4, KT, 512], BF16, tag="xsb")
            wqkx_sb = sb.tile([128, 3, KT, 128], BF16, tag="wqkx")
            wv_sb = sb.tile([128, KT, 192], BF16, tag="wv")
            misc_sb = sb.tile([128, 256], BF16, tag="misc")
            XOFF = 4 * KT * 512
            nc.sync.dma_start(misc_sb, misc[:, :])
            nc.sync.dma_start(
                wqkx_sb,
                xw[:, XOFF:XOFF + 3 * KT * 128].rearrange(
                    "p (w k m) -> p w k m", w=3, k=KT))
            xw_x = xw[:, 0:XOFF].rearrange("p (t k n) -> p t k n", t=4, k=KT)
            # nt0 split per-k so the first chain starts as k0 lands
            for k in range(KT):
                nc.sync.dma_start(xsb[:, 0, k], xw_x[:, 0, k])
            nc.sync.dma_start(
                wv_sb,
                xw[:, XOFF + 3 * KT * 128:XW].rearrange(
                    "p (k m) -> p k m", k=KT))
            for nt in range(1, 4):
                nc.sync.dma_start(xsb[:, nt], xw_x[:, nt])
            wo_sb = sb.tile([128, 2, D], BF16, tag="wo")
            nc.sync.dma_start(wo_sb, wo[:, :, :])
            trib_sb = misc_sb[:, 0:128]
            i128_sb = misc_sb[:, 128:256]
            wq_sb, wk_sb, wqk2_sb = (wqkx_sb[:, 0], wqkx_sb[:, 1],
                                     wqkx_sb[:, 2])
            wo01_sb = wo_sb[:, 0, :]
            wo2d_sb = wo_sb[:, 1, :]
            # preload the exp ACT table set (~1.3us) while DMA streams
            warm = sbn.tile([128, 64], F32, tag="warm", name="warm")
            nc.scalar.activation(warm[0:1, 0:2], misc_sb[0:1, 0:2],
                                 mybir.ActivationFunctionType.Exp, scale=1.0)

            # ---- phase 1: projections ----
            qt_sb = sb.tile([128, S], BF16, tag="qt")
            kt_sb = sb.tile([128, S], BF16, tag="kt")
            q2d = sb.tile([128, S], BF16, tag="q2d")
            k2d = sb.tile([128, S], BF16, tag="k2d")

            def _qk_casts(w, ntw, pp):
                if w == 0:
                    nc.vector.tensor_copy(qt_sb[:, ntw], pp)
                elif w == 1:
                    nc.vector.tensor_copy(kt_sb[:, ntw], pp)
                else:
                    nc.vector.tensor_copy(k2d[0:64, ntw], pp[0:64, :])
                    nc.vector.tensor_copy(q2d[64:128, ntw], pp[64:128, :])
                    nc.sync.dma_start(k2d[64:128, ntw], k2d[0:64, ntw])
                    nc.sync.dma_start(q2d[0:64, ntw], q2d[64:128, ntw])

            def proj_qk0():
                # 3 psum chains (3 bank-exclusive slots) accumulate per
                # arriving x k-tile so the PE starts ~2us into the DMA
                pa = pch.tile([128, 2, 512], F32, tag="sp", name="p0a")
                pb = pch.tile([128, 512], F32, tag="sp", name="p0b")
                slots = (pa[:, 0, :], pa[:, 1, :], pb)
                trips = (wq_sb, wk_sb, wqk2_sb)
                for k in range(KT):
                    for w, wsb in enumerate(trips):
                        nc.tensor.matmul(slots[w], wsb[:, k, :],
                                         xsb[:, 0, k, :],
                                         start=(k == 0), stop=(k == KT - 1))
                for w in range(3):
                    _qk_casts(w, slice(0, 512), slots[w])

            def filler_qk_units(nt):
                """proj_qk(nt) as 3 filler units (one 6-matmul chain each)."""
                ntw = slice(nt * 512, (nt + 1) * 512)

                def chain(w):
                    def f():
                        pp = ps.tile([128, 512], F32, tag="sc",
                                     name=f"pqk{nt}_{w}")
                        wsb = (wq_sb, wk_sb, wqk2_sb)[w]
                        for k in range(KT):
                            nc.tensor.matmul(pp, wsb[:, k, :],
                                             xsb[:, nt, k, :],
                                             start=(k == 0), stop=(k == KT - 1))
                        _qk_casts(w, ntw, pp)
                    return f
                return [(1.3, chain(w)) for w in range(3)]

            # v_aug: [128, NKV, 384]; head 0/2 at 128h: [v(64)|ones|zeros63];
            # head 1 reversed: [ones@0|zeros(63)|v(64)] so ctx1 lands in
            # psum rows 64:128 and den1 at row 0.
            vaug = sb.tile([128, NKV, 384], BF16, tag="vaug")
            for h in (0, 2):
                nc.vector.memset(vaug[:, :, 128 * h + 65:128 * (h + 1)], 0.0)
                nc.scalar.activation(
                    vaug[:, :, 128 * h + 64:128 * h + 65],
                    trib_sb[:, h * NKV:(h + 1) * NKV].rearrange(
                        "p (t c) -> p t c", c=1),
                    mybir.ActivationFunctionType.Copy, bias=1.0, scale=0.0)
            nc.vector.memset(vaug[:, :, 128 + 1:128 + 64], 0.0)
            nc.scalar.activation(
                vaug[:, :, 128 + 0:128 + 1],
                trib_sb[:, NKV:2 * NKV].rearrange("p (t c) -> p t c", c=1),
                mybir.ActivationFunctionType.Copy, bias=1.0, scale=0.0)

            def proj_v(i):
                """one filler unit: 6 matmuls of 192 cols + 2 casts
                (pp cols 0:64 -> v0; pp cols 64:192 -> v1|v2 contiguous)."""
                pp = ps.tile([128, 192], F32, tag="sc", name=f"pv{i}")
                c0 = (i % 4) * 128
                for k in range(KT):
                    nc.tensor.matmul(
                        pp,
                        xsb[:, i // 4, k, c0:c0 + 128],
                        wv_sb[:, k, :],
                        start=(k == 0), stop=(k == KT - 1))
                nc.vector.tensor_copy(vaug[:, i, 0:64], pp[:, 0:64])
                nc.vector.tensor_copy(vaug[:, i, 192:320], pp[:, 64:192])

            # ---- phase 2/3 helpers ----
            def kslc(h, i, g=None):
                if h == 0:
                    return kt_sb[0:64, i * 128:(i + 1) * 128]
                if h == 1:
                    return kt_sb[64:128, i * 128:(i + 1) * 128]
                return k2d[g:g + 64, i * 128:(i + 1) * 128]

            def qslc(h, c0, c1, g=None):
                if h == 0:
                    return qt_sb[0:64, c0:c1]
                if h == 1:
                    return qt_sb[64:128, c0:c1]
                return q2d[g:g + 64, c0:c1]

            ctxT01 = sb.tile([128, S], BF16, tag="ctxT01")
            ctxT2 = sb.tile([128, S], BF16, tag="ctxT2")

            def scores_pair(slots, J):
                """Paired bias+score emission for a list of (spt_h, h, i, g)
                covering DISTINCT psum banks.  One accumulation group per
                bank: bias starts it (pending-zeroing the bank), r0
                accumulates onto the bias, r1 lands on pending-zero bytes
                (= fresh write) and closes the group.  Emission interleaves
                the banks so the K=64 score mms pair on opposite row
                groups."""
                base = WJ * J
                diag = [s for s in slots if 128 * s[2] - base >= 0]
                full = [s for s in slots if 128 * s[2] - base < 0]
                for spt_h, h, i, g in full:
                    nc.tensor.matmul(spt_h, kslc(h, i, g=g),
                                     qslc(h, base, base + WJ, g=g),
                                     start=True, stop=True)
                for spt_h, h, i, g in diag:
                    c0 = 128 * i - base
                    nc.tensor.matmul(spt_h[:, c0:c0 + 128], trib_sb,
                                     i128_sb, start=True, stop=False)
                for spt_h, h, i, g in diag:
                    c0 = 128 * i - base
                    r1 = c0 + 128
                    nc.tensor.matmul(spt_h[:, c0:r1], kslc(h, i, g=g),
                                     qslc(h, base + c0, base + r1, g=g),
                                     start=False, stop=(r1 >= WJ))
                for spt_h, h, i, g in diag:
                    r1 = 128 * i - base + 128
                    if r1 < WJ:
                        nc.tensor.matmul(spt_h[:, r1:WJ], kslc(h, i, g=g),
                                         qslc(h, base + r1, base + WJ, g=g),
                                         start=False, stop=True)

            def chains01(J, fillers):
                """pass 1: heads 0+1, double-buffered scores."""
                ctx01 = psc.tile([128, 2, WJ], F32, tag="ctx",
                                 name=f"ctx01_{J}")
                imax = 4 * J + 3
                base = WJ * J

                def emit_ctx(i, esb2):
                    c0 = max(0, 128 * i - base)
                    for h in range(2):
                        nc.tensor.matmul(
                            ctx01[:, h, c0:WJ],
                            vaug[:, i, 128 * h:128 * h + 128],
                            esb2[:, h, c0:WJ],
                            start=(i == 0), stop=(i == imax))

                prev = None
                budget = 0.7
                for i in range(imax + 1):
                    c0 = max(0, 128 * i - base)
                    spt = pch.tile([128, 2, WJ], F32, tag="sp", name="spt2")
                    # h0 in bank0 (g0) || h1 in bank1 (g64)
                    scores_pair([(spt[:, 0, :], 0, i, None),
                                 (spt[:, 1, :], 1, i, None)], J)
                    esb2 = sbe.tile([128, 2, WJ], BF16, tag="exp2",
                                    name="esb2")
                    nc.scalar.activation(
                        esb2[:, :, c0:WJ], spt[:, :, c0:WJ],
                        mybir.ActivationFunctionType.Exp, scale=0.125)
                    if prev is not None:
                        emit_ctx(i - 1, prev)
                    prev = esb2
                    budget += 0.35
                    while fillers and budget >= fillers[0][0]:
                        cost, fn = fillers.pop(0)
                        budget -= cost
                        fn()
                emit_ctx(imax, prev)
                return ctx01

            def chains2(J, fillers):
                """pass 2: head 2, TWO kv tiles pipelined per step (the two
                pch bufs), paired g0||g64 via the dual q2d/k2d copies."""
                ctx2 = psc.tile([128, WJ], F32, tag="ctx", name=f"ctx2_{J}")
                imax = 4 * J + 3
                base = WJ * J

                def emit_ctx(i, esb1):
                    c0 = max(0, 128 * i - base)
                    nc.tensor.matmul(
                        ctx2[:, c0:WJ],
                        vaug[:, i, 256:384],
                        esb1[:, c0:WJ],
                        start=(i == 0), stop=(i == imax))

                prevs = []
                budget = 0.7
                for t in range((imax + 1) // 2):
                    i0, i1 = 2 * t, 2 * t + 1
                    sp0 = pch.tile([128, WJ], F32, tag="sp", name="spt1a")
                    sp1 = pch.tile([128, WJ], F32, tag="sp", name="spt1b")
                    scores_pair([(sp0, 2, i0, 0), (sp1, 2, i1, 64)], J)
                    for i, sp in ((i0, sp0), (i1, sp1)):
                        c0 = max(0, 128 * i - base)
                        esb1 = sbf.tile([128, WJ], BF16, tag="exp1",
                                        name="esb1")
                        nc.scalar.activation(
                            esb1[:, c0:WJ], sp[:, c0:WJ],
                            mybir.ActivationFunctionType.Exp, scale=0.125)
                        prevs.append((i, esb1))
                    while len(prevs) > 2:
                        emit_ctx(*prevs.pop(0))
                    budget += 0.45
                    while fillers and budget >= fillers[0][0]:
                        cost, fn = fillers.pop(0)
                        budget -= cost
                        fn()
                for pe in prevs:
                    emit_ctx(*pe)
                return ctx2

            def norms01(J, ctx01):
                w = slice(WJ * J, WJ * (J + 1))
                denT = sbn.tile([128, 2, WJ], BF16, tag="den", name="denT")
                nc.vector.tensor_copy(denT[64:65, 0, :], ctx01[64:65, 0, :])
                nc.vector.tensor_copy(denT[0:1, 0, :], ctx01[0:1, 1, :])
                bps0 = ps.tile([64, WJ], F32, tag="sc", name="bps0")
                nc.tensor.matmul(bps0, onesb[64:65, 0:64],
                                 denT[64:65, 0, :], start=True, stop=True)
                bps1 = ps.tile([64, WJ], F32, tag="sc", name="bps1")
                nc.tensor.matmul(bps1, onesb[0:1, 0:64],
                                 denT[0:1, 0, :], start=True, stop=True)
                inv0 = sbn.tile([64, WJ], F32, tag="inv", name="inv0")
                nc.vector.reciprocal_approx_fast(inv0, bps0)
                inv1 = sbn.tile([128, WJ], F32, tag="inv1", name="inv1")
                nc.vector.reciprocal_approx_fast(inv1[0:64, :], bps1)
                nc.sync.dma_start(inv1[64:128, :], inv1[0:64, :])
                nc.vector.tensor_mul(ctxT01[0:64, w], ctx01[0:64, 0, :],
                                     inv0)
                nc.vector.tensor_mul(ctxT01[64:128, w], ctx01[64:128, 1, :],
                                     inv1[64:128, :])

            def norms2(J, ctx2):
                w = slice(WJ * J, WJ * (J + 1))
                denT = sbn.tile([128, WJ], BF16, tag="den2", name="denT2")
                nc.vector.tensor_copy(denT[64:65, :], ctx2[64:65, :])
                bps2 = ps.tile([64, WJ], F32, tag="sc", name="bps2")
                nc.tensor.matmul(bps2, onesb[64:65, 0:64],
                                 denT[64:65, :], start=True, stop=True)
                inv2 = sbn.tile([64, WJ], F32, tag="inv2", name="inv2")
                nc.vector.reciprocal_approx_fast(inv2, bps2)
                nc.vector.tensor_mul(ctxT2[0:64, w], ctx2[0:64, :], inv2)
                # replicate ctxT2 into rows 64:128 for the g64 outproj pair
                nc.sync.dma_start(ctxT2[64:128, w], ctxT2[0:64, w])

            def outproj_units(J):
                """3 filler units; unit p = mt pair (2p, 2p+1): two K=128
                wo01 matmuls then the K=64 wo2 matmuls as a g0||g64 pair."""
                w = slice(WJ * J, WJ * (J + 1))

                def unit(p):
                    def f():
                        mta, mtb = 2 * p, 2 * p + 1
                        pa = ps.tile([128, WJ], F32, tag="sc", name="opA")
                        pb = ps.tile([128, WJ], F32, tag="sc", name="opB")
                        nc.tensor.matmul(
                            pa, wo01_sb[:, mta * 128:(mta + 1) * 128],
                            ctxT01[:, w], start=True, stop=False)
                        nc.tensor.matmul(
                            pb, wo01_sb[:, mtb * 128:(mtb + 1) * 128],
                            ctxT01[:, w], start=True, stop=False)
                        nc.tensor.matmul(
                            pa, wo2d_sb[0:64, mta * 128:(mta + 1) * 128],
                            ctxT2[0:64, w], start=False, stop=True)
                        nc.tensor.matmul(
                            pb, wo2d_sb[64:128, mtb * 128:(mtb + 1) * 128],
                            ctxT2[64:128, w], start=False, stop=True)
                        for mt, pp in ((mta, pa), (mtb, pb)):
                            osb = sbo.tile([128, WJ], F32, tag="osb",
                                           name="osb")
                            nc.vector.tensor_copy(osb, pp)
                            nc.sync.dma_start(
                                outT[mt * 128:(mt + 1) * 128, w], osb)
                    return f
                return [(1.1, unit(p)) for p in range(3)]

            # ones rows for the denominator broadcast (rows 0/64)
            onesb = sbn.tile([128, 64], BF16, tag="onesb", name="onesb")
            for r in (0, 64):
                nc.scalar.activation(
                    onesb[r:r + 1, :],
                    trib_sb[r:r + 1, 0:64],
                    mybir.ActivationFunctionType.Copy, bias=1.0, scale=0.0)

            # ---- interleaved schedule ----
            # qk(nt) is needed by window nt's pass1 - its filler units run
            # in window nt-1 and are flushed at the boundary.  proj_v(i)
            # must be emitted before the ctx matmuls that read vaug[i].
            proj_qk0()
            for i in range(4):
                proj_v(i)
            def window(J, fill1, fill2):
                c01 = chains01(J, fill1)
                fill2 = [(0.0, lambda: norms01(J, c01))] + fill1 + fill2
                c2 = chains2(J, fill2)
                for cost, fn in fill2:
                    fn()
                norms2(J, c2)

            op0 = outproj_units(0)
            op1 = outproj_units(1)
            op2 = outproj_units(2)
            window(0, filler_qk_units(1), [])
            for i in range(4, 8):
                proj_v(i)
            window(1, filler_qk_units(2), op0)
            for i in range(8, 12):
                proj_v(i)
            window(2, filler_qk_units(3), op1[0:1])
            for i in range(12, 16):
                proj_v(i)
            window(3, op1[1:] + op2[0:1], op2[1:])
            for cost, fn in outproj_units(3):
                fn()

    nc.compile()
    return nc


def shard_inputs(x, Wq, Wk, Wv, Wo):
    import ml_dtypes
    bf16 = ml_dtypes.bfloat16

    def krearrange(wT, cols):
        # [D, cols] -> [128, KT*cols]; line p holds wT[k*128+p, :] for all k
        return np.ascontiguousarray(
            wT.reshape(KT, 128, cols).transpose(1, 0, 2).reshape(128, KT * cols)
        ).astype(bf16)

    x = np.asarray(x, np.float32)
    trib = (np.triu(np.ones((128, 128), np.float32), 1) * NEG).astype(bf16)
    i128 = np.eye(128, dtype=np.float32).astype(bf16)
    misc_host = np.concatenate([trib, i128], axis=1)
    in_maps = []
    for c in range(NCORES):
        b, g = c // 4, c % 4
        rs = slice(GH * g, GH * g + GH)
        wqT = np.ascontiguousarray(np.asarray(Wq, np.float32)[rs].T)  # [D,192]
        wkT = np.ascontiguousarray(np.asarray(Wk, np.float32)[rs].T)
        wqk2 = np.concatenate([wkT[:, 128:192], wqT[:, 128:192]], axis=1)
        wv_t = np.ascontiguousarray(np.asarray(Wv, np.float32)[rs].T)  # [D,192]
        xT = np.ascontiguousarray(x[b].T)                     # [D, S]
        xb = xT.reshape(KT, 128, S)
        xparts = [np.ascontiguousarray(
                      xb[:, :, nt * 512:(nt + 1) * 512]
                  ).transpose(1, 0, 2).reshape(128, KT * 512)
                  for nt in range(4)]
        xw_host = np.concatenate(
            xparts + [krearrange(np.ascontiguousarray(wqT[:, 0:128]), 128),
                      krearrange(np.ascontiguousarray(wkT[:, 0:128]), 128),
                      krearrange(np.ascontiguousarray(wqk2), 128),
                      krearrange(wv_t, 192)], axis=1)
        woT = np.asarray(Wo, np.float32)[:, rs].T             # [192, 768]
        wo01 = woT[0:128]
        wo2d = np.concatenate([woT[128:192], woT[128:192]], axis=0)  # [128,768]
        wo_host = np.stack([wo01, wo2d], axis=1)              # [128, 2, 768]
        in_maps.append({
            "xw": np.ascontiguousarray(xw_host).astype(bf16),
            "wo": np.ascontiguousarray(wo_host).astype(bf16),
            "misc": misc_host,
        })
    return in_maps


def assemble(results, bo):
    out = np.zeros((B, S, D), np.float32)
    for c in range(NCORES):
        out[c // 4] += results[c]["outT"].T
    return out + np.asarray(bo, np.float32)[None, None, :]


_NC = None


def kernel(x, Wq, Wk, Wv, Wo, bo, **run_kwargs):
    global _NC
    if _NC is None:
        _NC = build()
    in_maps = shard_inputs(x, Wq, Wk, Wv, Wo)
    res = run_bass_kernel_spmd(_NC, in_maps, core_ids=list(range(NCORES)),
                               **run_kwargs)
    out = assemble(res.results, bo)
    kernel.last_results = res
    return out


# revision 24
# speedup vs baseline: 1.0408x; 1.0360x over previous
"""Trainium2 Bass kernel for causal multi-head attention (prefill).

Problem: x[2,2048,768], 12 heads x 64 dim, causal softmax(QK^T/8)V + out-proj.

Sharding (8 cores, no collectives): core c handles batch c//4 and head group
c%4 (3 heads).  Each core computes, for its batch b and heads hs:
    qT,kT = (Wq_hs @ x_b^T), (Wk_hs @ x_b^T)        [192, 2048] (transposed)
    v     = x_b @ Wv_hs^T                            [2048, 192]
    expT  = exp(scoresT/8) masked causally           [kv, sq] per head
    ctxT_h = v_aug^T @ expT  (extra row = softmax denom via ones column)
    outT_partial = Wo[:,cols_hs] @ (ctxT/den)        [768, 2048]
Host sums the 4 partial outputs per batch and transposes back.

v5 (rebuilt from v4 + trace analysis):
  - scores for all 3 heads land in ONE [128,3,512] psum tile (3 banks);
    a single merged ACT exp instruction reads all three banks, cutting
    per-instruction ACT overhead (~250ns each) by 3x.  The scores tile is
    single-buffered; PE idle during exp is filled with interleaved
    projection / out-projection "filler" units.
  - causal masking via a PE bias matmul (trib @ I adds -800 above the
    diagonal BEFORE score accumulation; exp then underflows to 0),
    removing the DVE tri-multiplies from the critical chain.
  - score matmuls are K=64: h0 rows 0:64 (g0), h1 rows 64:128 (g64), and
    h2 (dual-copied into both halves) is emitted as two column-halves on
    OPPOSITE row groups, so every score matmul runs 2x (concurrent tiles).
  - h1's v_aug weight is reversed ([zeros|ones@32|v]) so its ctx lands in
    psum rows 64:128 and its denominator at row 32: the normalize writes
    ctxT01[64:128] directly (no partition-shift DMA).
  - out-proj runs in mt-pairs: two K=128 wo01 matmuls then the two K=64
    wo2 matmuls as a concurrent g0/g64 pair (host packs wo2 twice).
  - bf16 datapath; PSUM + softmax normalization fp32.
"""

import numpy as np

import concourse.bass as bass
import concourse.tile as tile
from concourse import bacc, mybir
from concourse.bass_utils import run_bass_kernel_spmd

F32 = mybir.dt.float32
BF16 = mybir.dt.bfloat16

B, S, D = 2, 2048, 768
H, DH = 12, 64
HPC = 3                 # heads per core
GH = HPC * DH           # 192 head dims per core
NCORES = 8
KT = D // 128           # 6 contraction tiles for projections
WJ = 512                # q window width
NJ = S // WJ            # 4 windows
NKV = S // 128          # 16 kv tiles of 128
NEG = -30000.0          # causal bias: exp(0.125*(s+NEG)) == 0

MERGED_EXP = True       # one ACT instr over all 3 score banks (HW-validated)


def build():
    nc = bacc.Bacc("TRN2", target_bir_lowering=False, debug=False)

    # host pre-packed input: per partition line p:
    #   [ x (4 nt-blocks x 6 k x 512) | wq 6x128 | wk 6x128 | wqk2 6x128
    #     | wv 6x192 ]  (wqk2 cols 0:64 = Wk2^T, 64:128 = Wq2^T)
    XW = 4 * KT * 512 + 3 * KT * 128 + KT * 192
    xw = nc.dram_tensor("xw", [128, XW], BF16, kind="ExternalInput")
    # wo[:, 0, :] = Wo cols (head rows) 0:128; wo[:, 1, :] = rows 128:192
    # duplicated twice (p and p-64 hold the same row) for g0/g64 pairing.
    wo = nc.dram_tensor("wo", [128, 2, D], BF16, kind="ExternalInput")
    # misc[:, 0:128] = trib (NEG above diagonal, [sq, kv] layout);
    # misc[:, 128:256] = identity.
    misc = nc.dram_tensor("misc", [128, 256], BF16, kind="ExternalInput")
    outT = nc.dram_tensor("outT", [D, S], F32, kind="ExternalOutput")

    with tile.TileContext(nc) as tc, \
         nc.allow_low_precision(reason="bf16 datapath, fp32 psum/normalize"):
        with tc.tile_pool(name="sb", bufs=1) as sb, \
             tc.tile_pool(name="sbe", bufs=2) as sbe, \
             tc.tile_pool(name="sbo", bufs=3) as sbo, \
             tc.tile_pool(name="sbn", bufs=3) as sbn, \
             tc.tile_pool(name="ps", bufs=2, space="PSUM") as ps, \
             tc.tile_pool(name="pch", bufs=1, space="PSUM") as pch, \
             tc.tile_pool(name="psc", bufs=3, space="PSUM") as psc:

            # ---- phase 0: load weights + x ----
            xsb = sb.tile([128, 4, KT, 512], BF16, tag="xsb")
            wqkx_sb = sb.tile([128, 3, KT, 128], BF16, tag="wqkx")
            wv_sb = sb.tile([128, KT, 192], BF16, tag="wv")
            misc_sb = sb.tile([128, 256], BF16, tag="misc")
            XOFF = 4 * KT * 512
            nc.sync.dma_start(misc_sb, misc[:, :])
            nc.sync.dma_start(
                wqkx_sb,
                xw[:, XOFF:XOFF + 3 * KT * 128].rearrange(
                    "p (w k m) -> p w k m", w=3, k=KT))
            xw_x = xw[:, 0:XOFF].rearrange("p (t k n) -> p t k n", t=4, k=KT)
            # nt0 split per-k so the first chain starts as k0 lands
            for k in range(KT):
                nc.sync.dma_start(xsb[:, 0, k], xw_x[:, 0, k])
            nc.sync.dma_start(
                wv_sb,
                xw[:, XOFF + 3 * KT * 128:XW].rearrange(
                    "p (k m) -> p k m", k=KT))
            for nt in range(1, 4):
                nc.sync.dma_start(xsb[:, nt], xw_x[:, nt])
            wo_sb = sb.tile([128, 2, D], BF16, tag="wo")
            nc.sync.dma_start(wo_sb, wo[:, :, :])
            trib_sb = misc_sb[:, 0:128]
            i128_sb = misc_sb[:, 128:256]
            wq_sb, wk_sb, wqk2_sb = (wqkx_sb[:, 0], wqkx_sb[:, 1],
                                     wqkx_sb[:, 2])
            wo01_sb = wo_sb[:, 0, :]
            wo2d_sb = wo_sb[:, 1, :]
            # preload the exp ACT table set (~1.3us) while DMA streams
            warm = sbn.tile([128, 64], F32, tag="warm", name="warm")
            nc.scalar.activation(warm[0:1, 0:2], misc_sb[0:1, 0:2],
                                 mybir.ActivationFunctionType.Exp, scale=1.0)

            # ---- phase 1: projections ----
            # heads 0/1: qt/kt [128, S] (h0 rows 0:64, h1 rows 64:128).
            # head 2: dual-group tiles q2d/k2d [128, S] - the same 64 rows
            # replicated in both halves so scores can alternate row-groups.
            qt_sb = sb.tile([128, S], BF16, tag="qt")
            kt_sb = sb.tile([128, S], BF16, tag="kt")
            q2d = sb.tile([128, S], BF16, tag="q2d")
            k2d = sb.tile([128, S], BF16, tag="k2d")

            def proj_qk0():
                # first window: 3 psum chains (the pch banks) accumulate
                # per arriving x k-tile so PE starts ~2us in
                ntw = slice(0, 512)
                trips = (wq_sb, wk_sb, wqk2_sb)
                pp3 = pch.tile([128, 3, 512], F32, tag="sp", name="pp3")
                for k in range(KT):
                    for w, wsb in enumerate(trips):
                        nc.tensor.matmul(pp3[:, w, :], wsb[:, k, :],
                                         xsb[:, 0, k, :],
                                         start=(k == 0), stop=(k == KT - 1))
                nc.vector.tensor_copy(qt_sb[:, ntw], pp3[:, 0, :])
                nc.vector.tensor_copy(kt_sb[:, ntw], pp3[:, 1, :])
                nc.vector.tensor_copy(k2d[0:64, ntw], pp3[0:64, 2, :])
                nc.vector.tensor_copy(q2d[64:128, ntw], pp3[64:128, 2, :])
                nc.sync.dma_start(k2d[64:128, ntw], k2d[0:64, ntw])
                nc.sync.dma_start(q2d[0:64, ntw], q2d[64:128, ntw])

            def filler_qk_units(nt):
                """proj_qk(nt) as 3 filler units (one 6-matmul chain each)."""
                ntw = slice(nt * 512, (nt + 1) * 512)

                def chain(w):
                    def f():
                        pp = ps.tile([128, 512], F32, tag="sc",
                                     name=f"pqk{nt}_{w}")
                        wsb = (wq_sb, wk_sb, wqk2_sb)[w]
                        for k in range(KT):
                            nc.tensor.matmul(pp, wsb[:, k, :],
                                             xsb[:, nt, k, :],
                                             start=(k == 0), stop=(k == KT - 1))
                        if w == 0:
                            nc.vector.tensor_copy(qt_sb[:, ntw], pp)
                        elif w == 1:
                            nc.vector.tensor_copy(kt_sb[:, ntw], pp)
                        else:
                            nc.vector.tensor_copy(k2d[0:64, ntw], pp[0:64, :])
                            nc.vector.tensor_copy(q2d[64:128, ntw],
                                                  pp[64:128, :])
                            nc.sync.dma_start(k2d[64:128, ntw],
                                              k2d[0:64, ntw])
                            nc.sync.dma_start(q2d[0:64, ntw],
                                              q2d[64:128, ntw])
                    return f
                return [(1.3, chain(w)) for w in range(3)]

            # v_aug: [128, NKV, 384]; head 0/2 at 128h: [v(64)|ones|zeros63];
            # head 1 reversed: [ones@0|zeros(63)|v(64)] so ctx1 lands in
            # psum rows 64:128 and den1 at row 0.
            vaug = sb.tile([128, NKV, 384], BF16, tag="vaug")
            for h in (0, 2):
                nc.vector.memset(vaug[:, :, 128 * h + 65:128 * (h + 1)], 0.0)
                nc.scalar.activation(
                    vaug[:, :, 128 * h + 64:128 * h + 65],
                    trib_sb[:, h * NKV:(h + 1) * NKV].rearrange(
                        "p (t c) -> p t c", c=1),
                    mybir.ActivationFunctionType.Copy, bias=1.0, scale=0.0)
            nc.vector.memset(vaug[:, :, 128 + 1:128 + 64], 0.0)
            nc.scalar.activation(
                vaug[:, :, 128 + 0:128 + 1],
                trib_sb[:, NKV:2 * NKV].rearrange("p (t c) -> p t c", c=1),
                mybir.ActivationFunctionType.Copy, bias=1.0, scale=0.0)

            def proj_v(i):
                """one filler unit: 6 matmuls of 192 cols + cast."""
                pp = ps.tile([128, 192], F32, tag="sc", name=f"pv{i}")
                c0 = (i % 4) * 128
                for k in range(KT):
                    nc.tensor.matmul(
                        pp,
                        xsb[:, i // 4, k, c0:c0 + 128],
                        wv_sb[:, k, :],
                        start=(k == 0), stop=(k == KT - 1))
                vv = vaug[:, i, :].rearrange("p (h c) -> p h c", c=128)
                ppv = pp.rearrange("p (h c) -> p h c", c=64)
                nc.vector.tensor_copy(vv[:, 0, 0:64], ppv[:, 0])
                nc.vector.tensor_copy(vv[:, 1, 64:128], ppv[:, 1])
                nc.vector.tensor_copy(vv[:, 2, 0:64], ppv[:, 2])

            # ---- phase 2/3 helpers ----
            def kslc(h, i, g=None):
                if h == 0:
                    return kt_sb[0:64, i * 128:(i + 1) * 128]
                if h == 1:
                    return kt_sb[64:128, i * 128:(i + 1) * 128]
                return k2d[g:g + 64, i * 128:(i + 1) * 128]

            def qslc(h, c0, c1, g=None):
                if h == 0:
                    return qt_sb[0:64, c0:c1]
                if h == 1:
                    return qt_sb[64:128, c0:c1]
                return q2d[g:g + 64, c0:c1]

            ctxT01 = sb.tile([128, S], BF16, tag="ctxT01")
            ctxT2 = sb.tile([128, S], BF16, tag="ctxT2")

            def chains(J, fillers):
                """Per kv-tile i: [diag bias mms] -> score mms (all paired)
                -> merged exp -> ctx(i) after exp; fillers keep the PE fed
                while ACT computes exp."""
                ctxp = [psc.tile([128, WJ], F32, tag="ctx", name=f"ctx{J}_{h}")
                        for h in range(HPC)]
                imax = 4 * J + 3
                base = WJ * J

                def emit_ctx(i, esb3):
                    c0 = max(0, 128 * i - base)
                    for h in range(HPC):
                        nc.tensor.matmul(
                            ctxp[h][:, c0:WJ],
                            vaug[:, i, 128 * h:128 * h + 128],
                            esb3[:, h, c0:WJ],
                            start=(i == 0), stop=(i == imax))

                prev = None
                budget = 0.0
                for i in range(imax + 1):
                    d = 128 * i - base
                    c0 = max(0, d)
                    spt = pch.tile([128, 3, WJ], F32, tag="sp", name="spt")
                    if d >= 0:
                        # diagonal tile: causal bias then split-region scores
                        for h in range(HPC):
                            nc.tensor.matmul(spt[:, h, c0:c0 + 128],
                                             trib_sb, i128_sb,
                                             start=True, stop=False)
                        r1 = c0 + 128  # start of fully-visible region
                        nc.tensor.matmul(spt[:, 0, c0:r1], kslc(0, i),
                                         qslc(0, base + c0, base + r1),
                                         start=False, stop=True)
                        nc.tensor.matmul(spt[:, 1, c0:r1], kslc(1, i),
                                         qslc(1, base + c0, base + r1),
                                         start=False, stop=True)
                        nc.tensor.matmul(spt[:, 2, c0:r1], kslc(2, i, g=0),
                                         qslc(2, base + c0, base + r1, g=0),
                                         start=False, stop=True)
                        if r1 < WJ:
                            nc.tensor.matmul(spt[:, 0, r1:WJ], kslc(0, i),
                                             qslc(0, base + r1, base + WJ),
                                             start=True, stop=True)
                            nc.tensor.matmul(spt[:, 1, r1:WJ], kslc(1, i),
                                             qslc(1, base + r1, base + WJ),
                                             start=True, stop=True)
                            nc.tensor.matmul(spt[:, 2, r1:WJ],
                                             kslc(2, i, g=64),
                                             qslc(2, base + r1, base + WJ,
                                                  g=64),
                                             start=True, stop=True)
                    else:
                        # full tile: h2 is split into two col-halves on
                        # opposite row groups; order [h2a(g64), h0(g0),
                        # h1(g64), h2b(g0)] keeps perfect g64/g0 alternation
                        # (2x tile concurrency) while the two bank-2 writes
                        # stay time-disjoint (no same-bank write collision).
                        mid = WJ // 2
                        nc.tensor.matmul(spt[:, 2, 0:mid], kslc(2, i, g=64),
                                         qslc(2, base, base + mid, g=64),
                                         start=True, stop=True)
                        nc.tensor.matmul(spt[:, 0, :], kslc(0, i),
                                         qslc(0, base, base + WJ),
                                         start=True, stop=True)
                        nc.tensor.matmul(spt[:, 1, :], kslc(1, i),
                                         qslc(1, base, base + WJ),
                                         start=True, stop=True)
                        nc.tensor.matmul(spt[:, 2, mid:WJ], kslc(2, i, g=0),
                                         qslc(2, base + mid, base + WJ, g=0),
                                         start=True, stop=True)
                    esb3 = sbe.tile([128, HPC, WJ], BF16, tag="exp",
                                    name="esb3")
                    if MERGED_EXP:
                        nc.scalar.activation(
                            esb3[:, :, c0:WJ], spt[:, :, c0:WJ],
                            mybir.ActivationFunctionType.Exp, scale=0.125)
                    else:
                        for h in range(HPC):
                            nc.scalar.activation(
                                esb3[:, h, c0:WJ], spt[:, h, c0:WJ],
                                mybir.ActivationFunctionType.Exp, scale=0.125)
                    if prev is not None:
                        emit_ctx(i - 1, prev)
                    prev = esb3
                    # filler units to cover the exp latency
                    budget += 0.55
                    while fillers and budget >= fillers[0][0]:
                        cost, fn = fillers.pop(0)
                        budget -= cost
                        fn()
                emit_ctx(imax, prev)
                return ctxp

            def norms(J, ctxp):
                """normalize by the softmax denominator (psum row 64 for
                h0/h2, row 32 for h1).  Broadcast via K=1 matmuls (paired
                row positions 64 / 32); recip + final mul on DVE."""
                w = slice(WJ * J, WJ * (J + 1))
                denT = sbn.tile([128, 2, WJ], BF16, tag="den", name="denT")
                nc.vector.tensor_copy(denT[64:65, 0, :], ctxp[0][64:65, :])
                nc.vector.tensor_copy(denT[0:1, 0, :], ctxp[1][0:1, :])
                nc.vector.tensor_copy(denT[64:65, 1, :], ctxp[2][64:65, :])
                bps0 = ps.tile([64, WJ], F32, tag="sc", name="bps0")
                nc.tensor.matmul(bps0, onesb[64:65, 0:64],
                                 denT[64:65, 0, :], start=True, stop=True)
                bps1 = ps.tile([64, WJ], F32, tag="sc", name="bps1")
                nc.tensor.matmul(bps1, onesb[0:1, 0:64],
                                 denT[0:1, 0, :], start=True, stop=True)
                bps2 = ps.tile([64, WJ], F32, tag="sc", name="bps2")
                nc.tensor.matmul(bps2, onesb[64:65, 0:64],
                                 denT[64:65, 1, :], start=True, stop=True)
                inv0 = sbn.tile([64, WJ], F32, tag="inv", name="inv0")
                nc.vector.reciprocal_approx_fast(inv0, bps0)
                inv1 = sbn.tile([128, WJ], F32, tag="inv1", name="inv1")
                nc.vector.reciprocal_approx_fast(inv1[0:64, :], bps1)
                # shift h1's reciprocal into rows 64:128 (DVE is
                # partition-aligned; ctx1 lives in psum rows 64:128)
                nc.sync.dma_start(inv1[64:128, :], inv1[0:64, :])
                inv2 = sbn.tile([64, WJ], F32, tag="inv2", name="inv2")
                nc.vector.reciprocal_approx_fast(inv2, bps2)
                nc.vector.tensor_mul(ctxT01[0:64, w], ctxp[0][0:64, :], inv0)
                nc.vector.tensor_mul(ctxT01[64:128, w], ctxp[1][64:128, :],
                                     inv1[64:128, :])
                nc.vector.tensor_mul(ctxT2[0:64, w], ctxp[2][0:64, :], inv2)
                # replicate ctxT2 into rows 64:128 for the g64 outproj pair
                nc.sync.dma_start(ctxT2[64:128, w], ctxT2[0:64, w])

            def outproj_units(J):
                """3 filler units; unit p = mt pair (2p, 2p+1): two K=128
                wo01 matmuls then the K=64 wo2 matmuls as a g0||g64 pair."""
                w = slice(WJ * J, WJ * (J + 1))

                def unit(p):
                    def f():
                        mta, mtb = 2 * p, 2 * p + 1
                        pa = ps.tile([128, WJ], F32, tag="sc", name="opA")
                        pb = ps.tile([128, WJ], F32, tag="sc", name="opB")
                        nc.tensor.matmul(
                            pa, wo01_sb[:, mta * 128:(mta + 1) * 128],
                            ctxT01[:, w], start=True, stop=False)
                        nc.tensor.matmul(
                            pb, wo01_sb[:, mtb * 128:(mtb + 1) * 128],
                            ctxT01[:, w], start=True, stop=False)
                        nc.tensor.matmul(
                            pa, wo2d_sb[0:64, mta * 128:(mta + 1) * 128],
                            ctxT2[0:64, w], start=False, stop=True)
                        nc.tensor.matmul(
                            pb, wo2d_sb[64:128, mtb * 128:(mtb + 1) * 128],
                            ctxT2[64:128, w], start=False, stop=True)
                        for mt, pp in ((mta, pa), (mtb, pb)):
                            osb = sbo.tile([128, WJ], F32, tag="osb",
                                           name="osb")
                            nc.vector.tensor_copy(osb, pp)
                            nc.sync.dma_start(
                                outT[mt * 128:(mt + 1) * 128, w], osb)
                    return f
                return [(1.1, unit(p)) for p in range(3)]

            # ones rows for the denominator broadcast (rows 0/64)
            onesb = sbn.tile([128, 64], BF16, tag="onesb", name="onesb")
            for r in (0, 64):
                nc.scalar.activation(
                    onesb[r:r + 1, :],
                    trib_sb[r:r + 1, 0:64],
                    mybir.ActivationFunctionType.Copy, bias=1.0, scale=0.0)

            # ---- interleaved schedule ----
            proj_qk0()
            for i in range(4):
                proj_v(i)
            f0 = filler_qk_units(1)
            ctx0 = chains(0, f0)
            for cost, fn in f0:
                fn()
            f0.clear()
            for i in range(4, 8):
                proj_v(i)
            norms(0, ctx0)
            f1 = filler_qk_units(2)
            ctx1 = chains(1, f1)
            for cost, fn in f1:
                fn()
            f1.clear()
            for i in range(8, 12):
                proj_v(i)
            norms(1, ctx1)
            f2 = filler_qk_units(3) + outproj_units(0)
            ctx2 = chains(2, f2)
            for cost, fn in f2:
                fn()
            f2.clear()
            norms(2, ctx2)
            f3 = ([(0.75, lambda i=i: proj_v(i)) for i in range(12, 16)]
                  + outproj_units(1) + outproj_units(2))
            ctx3 = chains(3, f3)
            for cost, fn in f3:
                fn()
            f3.clear()
            norms(3, ctx3)
            for cost, fn in outproj_units(3):
                fn()

    nc.compile()
    return nc


def shard_inputs(x, Wq, Wk, Wv, Wo):
    import ml_dtypes
    bf16 = ml_dtypes.bfloat16

    def krearrange(wT, cols):
        # [D, cols] -> [128, KT*cols]; line p holds wT[k*128+p, :] for all k
        return np.ascontiguousarray(
            wT.reshape(KT, 128, cols).transpose(1, 0, 2).reshape(128, KT * cols)
        ).astype(bf16)

    x = np.asarray(x, np.float32)
    trib = (np.triu(np.ones((128, 128), np.float32), 1) * NEG).astype(bf16)
    i128 = np.eye(128, dtype=np.float32).astype(bf16)
    misc_host = np.concatenate([trib, i128], axis=1)
    in_maps = []
    for c in range(NCORES):
        b, g = c // 4, c % 4
        rs = slice(GH * g, GH * g + GH)
        wqT = np.ascontiguousarray(np.asarray(Wq, np.float32)[rs].T)  # [D,192]
        wkT = np.ascontiguousarray(np.asarray(Wk, np.float32)[rs].T)
        wqk2 = np.concatenate([wkT[:, 128:192], wqT[:, 128:192]], axis=1)
        wv_t = np.ascontiguousarray(np.asarray(Wv, np.float32)[rs].T)  # [D,192]
        xT = np.ascontiguousarray(x[b].T)                     # [D, S]
        xb = xT.reshape(KT, 128, S)
        xparts = [np.ascontiguousarray(
                      xb[:, :, nt * 512:(nt + 1) * 512]
                  ).transpose(1, 0, 2).reshape(128, KT * 512)
                  for nt in range(4)]
        xw_host = np.concatenate(
            xparts + [krearrange(np.ascontiguousarray(wqT[:, 0:128]), 128),
                      krearrange(np.ascontiguousarray(wkT[:, 0:128]), 128),
                      krearrange(np.ascontiguousarray(wqk2), 128),
                      krearrange(wv_t, 192)], axis=1)
        woT = np.asarray(Wo, np.float32)[:, rs].T             # [192, 768]
        wo01 = woT[0:128]
        wo2d = np.concatenate([woT[128:192], woT[128:192]], axis=0)  # [128,768]
        wo_host = np.stack([wo01, wo2d], axis=1)              # [128, 2, 768]
        in_maps.append({
            "xw": np.ascontiguousarray(xw_host).astype(bf16),
            "wo": np.ascontiguousarray(wo_host).astype(bf16),
            "misc": misc_host,
        })
    return in_maps


def assemble(results, bo):
    out = np.zeros((B, S, D), np.float32)
    for c in range(NCORES):
        out[c // 4] += results[c]["outT"].T
    return out + np.asarray(bo, np.float32)[None, None, :]


_NC = None


def kernel(x, Wq, Wk, Wv, Wo, bo, **run_kwargs):
    global _NC
    if _NC is None:
        _NC = build()
    in_maps = shard_inputs(x, Wq, Wk, Wv, Wo)
    res = run_bass_kernel_spmd(_NC, in_maps, core_ids=list(range(NCORES)),
                               **run_kwargs)
    out = assemble(res.results, bo)
    kernel.last_results = res
    return out
